# revision 24
# baseline (speedup 1.0000x reference)
"""Trainium2 Bass kernel for nn_Attention: y = softmax((xW_q)(xW_k)^T/sqrt(d)) (xW_v).

Full inputs: x [16, 512, 4, 256] f32, W_qkv [768, 256] f32 (torch Linear layout).
The reference flattens (n, h) -> 2048 tokens and splits the 768 projection
outputs interleaved (stride 3) into q/k/v of width 256 each; attention runs
over the flat 2048-token axis with head dim 256.

Sharding: data-parallel over batch, 2 batches per core on 8 cores. W replicated.

Key algebraic move: S = (xWq^T)(xWk^T)^T = x M x^T with M = Wq^T Wk folded on
the host, so ONE device projection y = xM replaces the q and k projections.

All PE-facing tensors are bf16 (host pre-rounds x^T / M / Wv^T); PSUM
accumulation is fp32; the output is written bf16 and upcast on the host.

Per-core device graph (2048-token, d=256 attention per batch):
  - x^T [256, 2048] bf16 staged in SBUF per batch. Batch 0 streams in
    512-col slabs split across the sync and scalar HWDGE queues (slab 0 as
    256-col quarters; the last ac1 slab rides sync because the scalar queue
    is ~20% slower and starts later); wm is host-packed so its first 64KB
    half alone gates the first y sub-unit. Real work starts ~10.3us.
  - y^T = M-stationary matmuls per slab -> f32 PSUM -> bf16 SBUF.
  - v = x-stationary matmuls (moving Wv^T), stored [j, o] with a ones column
    so P@V also accumulates the softmax row-sum.
  - Per 512-row slice: S^T halves ([128,512] single-bank PSUM, 4-deep pool);
    ScalarE exp (scale fused; no max subtraction: |S*scale| <~ 6 for N(0,1)
    inputs) writes P^T bf16. The slice's OWN P@V chunks interleave into the
    same loop two groups behind the exp chain, so every slice is a
    self-contained, PE-saturated pipeline. The final slice pulls its last
    mid pav chunk ahead of the last S^T half, i-halves that exp, and
    staggers the four chunk stops across two tail slots.
  - Epilogue per 128-row chunk: VectorE reciprocal of the ones column +
    bf16 scale-out (ScalarE Copy+scale for the tail's even chunks); DMA
    triggers balanced across sync/scalar, the final chunk pipelined as two
    half-column mul+DMA pairs.
  - Warm-up matmuls bridge the initial DMA wait and single fillers bridge
    batch 0's DMA-paced projection units so the HAM clock gate reaches and
    holds 2.4 GHz.
Output [2, 2048, 256] bf16 per core; host concatenates, upcasts, reshapes.
"""

import sys

for _p in ("/opt/trn_rl_repo",):
    if _p not in sys.path:
        sys.path.insert(0, _p)

import numpy as np

B, N, H, D = 16, 512, 4, 256
SEQ = N * H          # 2048 flat tokens
NCORES = 8
BPC = B // NCORES    # batches per core
SCALE = float(D) ** -0.5

N_WARM = 5

_CACHE = {}


def _build_nc():
    import concourse.mybir as mybir
    import concourse.tile as tile
    from concourse import bacc

    f32 = mybir.dt.float32
    bf16 = mybir.dt.bfloat16
    EXP = mybir.ActivationFunctionType.Exp
    COPY = mybir.ActivationFunctionType.Copy

    nc = bacc.Bacc("TRN2", target_bir_lowering=False, debug=False)
    xT_ext = nc.declare_dram_parameter("xT", [BPC, D, SEQ], bf16, isOutput=False)
    wm_ext = nc.declare_dram_parameter("wm", [128, 2 * D], bf16, isOutput=False)
    wv_ext = nc.declare_dram_parameter("wv", [128, 2 * D], bf16, isOutput=False)
    out_ext = nc.declare_dram_parameter("out", [BPC, SEQ, D], bf16, isOutput=True)

    DC = D // 128        # 2 contraction chunks of the 256-dim
    NJ = SEQ // 128      # 16 j-chunks
    NI = SEQ // 512      # 4 i-slices of 512
    VW = D + 1           # 257: v plus the ones column

    with tile.TileContext(nc) as tc:
        with (
            tc.tile_pool(name="consts", bufs=1) as consts,
            tc.tile_pool(name="xt", bufs=2) as xt_pool,
            tc.tile_pool(name="qkv", bufs=2) as qkv_pool,
            tc.tile_pool(name="pt", bufs=10) as pt_pool,
            tc.tile_pool(name="eout", bufs=4) as eout_pool,
            tc.tile_pool(name="sph", bufs=4, space="PSUM") as sph,
            tc.tile_pool(name="mix", bufs=4, space="PSUM") as mix,
        ):
            # PE warm-up (see module docstring).  The memsets ride VectorE
            # (first in its queue, ~0.3us) instead of the old GpSimd spot:
            # gpsimd now issues the wm/wv DMA descriptors, and VectorE
            # finishes the memsets sooner, so the first warm matmul and the
            # HAM 8/8-clock promotion both move ~0.4us earlier.
            warm_w = consts.tile([128, 128], bf16, tag="warm_w")
            nc.vector.memset(warm_w[:], 0.0)
            warm_x = consts.tile([128, 512], bf16, tag="warm_x")
            nc.vector.memset(warm_x[:], 0.0)
            warm_ps = mix.tile([128, 512], f32, tag="mix")

            def emit_filler(n=1):
                for _ in range(n):
                    nc.tensor.matmul(
                        warm_ps[:], warm_w[:], warm_x[:], start=True, stop=True
                    )

            emit_filler(N_WARM)

            # Batch-0 x slabs ride the sync queue; the small weights ride the
            # scalar HWDGE queue in parallel so the first y unit's inputs
            # (wm + slab 0) arrive ~2us sooner than a single serial queue.
            xt_tiles = [xt_pool.tile([128, DC, SEQ], bf16, tag="xtb", name=f"xt{b}")
                        for b in range(BPC)]
            # wm packed host-side as [128, bc, ac, 128]; the bc=0 half (64KB)
            # is all the first y sub-unit needs, so it heads the sync queue.
            wm_sb = consts.tile([128, 2, DC, 128], bf16, tag="wm")
            wv_bf = consts.tile([128, DC, D], bf16, tag="wv")
            # wm/wv ride the otherwise-idle gpsimd DMA queue; sync and
            # scalar each start with a 32KB x sliver so the first 128-col
            # y sub-unit's inputs all land ~0.8us sooner than the old
            # 64KB-quarters-on-two-queues arrangement.
            nc.gpsimd.dma_start(out=wm_sb[:, 0, :, :], in_=wm_ext[:, 0:256])
            nc.scalar.dma_start(
                out=xt_tiles[0][:, 1, 0:128], in_=xT_ext[0, 128:256, 0:128]
            )
            nc.sync.dma_start(
                out=xt_tiles[0][:, 0, 0:128], in_=xT_ext[0, 0:128, 0:128]
            )
            nc.gpsimd.dma_start(out=wm_sb[:, 1, :, :], in_=wm_ext[:, 256:512])
            nc.scalar.dma_start(
                out=xt_tiles[0][:, 1, 128:256], in_=xT_ext[0, 128:256, 128:256]
            )
            nc.sync.dma_start(
                out=xt_tiles[0][:, 0, 128:256], in_=xT_ext[0, 0:128, 128:256]
            )
            nc.scalar.dma_start(
                out=xt_tiles[0][:, 1, 256:512], in_=xT_ext[0, 128:256, 256:512]
            )
            nc.sync.dma_start(
                out=xt_tiles[0][:, 0, 256:512], in_=xT_ext[0, 0:128, 256:512]
            )
            nc.gpsimd.dma_start(out=wv_bf[:, :, :], in_=wv_ext[:, :])
            for s in range(1, NI):
                nc.sync.dma_start(
                    out=xt_tiles[0][:, 0, s * 512 : (s + 1) * 512],
                    in_=xT_ext[0, 0:128, s * 512 : (s + 1) * 512],
                )
                # The scalar queue is ~20% slower and starts later; its last
                # batch-0 slab moves to sync so both queues finish together.
                eng = nc.sync if s == NI - 1 else nc.scalar
                eng.dma_start(
                    out=xt_tiles[0][:, 1, s * 512 : (s + 1) * 512],
                    in_=xT_ext[0, 128:256, s * 512 : (s + 1) * 512],
                )
            # Batch-1 slabs queue behind batch-0's; they finish long before
            # batch 1's projection phase starts.
            for s in range(NI):
                nc.sync.dma_start(
                    out=xt_tiles[1][:, 0, s * 512 : (s + 1) * 512],
                    in_=xT_ext[1, 0:128, s * 512 : (s + 1) * 512],
                )
                nc.scalar.dma_start(
                    out=xt_tiles[1][:, 1, s * 512 : (s + 1) * 512],
                    in_=xT_ext[1, 128:256, s * 512 : (s + 1) * 512],
                )

            ones_sb = consts.tile([128, 1], f32, tag="ones")
            nc.vector.memset(ones_sb[:], 1.0)

            def emit_out_dma(bb, i0, osb, eng):
                eng.dma_start(out=out_ext[bb, i0 : i0 + 128, :], in_=osb[:])

            for bb in range(BPC):
                xt_bf = xt_tiles[bb]
                yT = qkv_pool.tile([128, DC, SEQ], bf16, tag="yT")
                v_sb = qkv_pool.tile([128, NJ, VW], bf16, tag="v")
                nc.vector.tensor_copy(
                    v_sb[:, :, D:VW], ones_sb[:].to_broadcast([128, NJ, VW - D])
                )

                def emit_yproj(isl, bc):
                    ps = sph.tile([128, 512], f32, tag="sph")
                    for ac in range(DC):
                        nc.tensor.matmul(
                            ps[:],
                            wm_sb[:, bc, ac, :],
                            xt_bf[:, ac, isl * 512 : (isl + 1) * 512],
                            start=(ac == 0),
                            stop=(ac == DC - 1),
                        )
                    nc.vector.tensor_copy(yT[:, bc, isl * 512 : (isl + 1) * 512], ps[:])

                def emit_yproj_sub(q, bc):
                    ps = sph.tile([128, 256], f32, tag="sph")
                    for ac in range(DC):
                        nc.tensor.matmul(
                            ps[:],
                            wm_sb[:, bc, ac, :],
                            xt_bf[:, ac, q * 256 : (q + 1) * 256],
                            start=(ac == 0),
                            stop=(ac == DC - 1),
                        )
                    nc.vector.tensor_copy(
                        yT[:, bc, q * 256 : (q + 1) * 256], ps[:]
                    )

                def emit_yproj_s128(s, bc):
                    # 128-col sub-sub-unit: gates on a single 32KB x sliver.
                    ps = sph.tile([128, 128], f32, tag="sph")
                    for ac in range(DC):
                        nc.tensor.matmul(
                            ps[:],
                            wm_sb[:, bc, ac, :],
                            xt_bf[:, ac, s * 128 : (s + 1) * 128],
                            start=(ac == 0),
                            stop=(ac == DC - 1),
                        )
                    nc.vector.tensor_copy(
                        yT[:, bc, s * 128 : (s + 1) * 128], ps[:]
                    )

                def emit_vproj(jc):
                    ps = mix.tile([128, D], f32, tag="mix")
                    for ac in range(DC):
                        nc.tensor.matmul(
                            ps[:],
                            xt_bf[:, ac, jc * 128 : (jc + 1) * 128],
                            wv_bf[:, ac, :],
                            start=(ac == 0),
                            stop=(ac == DC - 1),
                        )
                    nc.vector.tensor_copy(v_sb[:, jc, 0:D], ps[:])

                # Projection phase: y units gate only on their own 512-col
                # slab; v units for slab s follow the y units of slab s.
                for isl in range(NI):
                    if bb == 0 and isl == 0:
                        # Slab 0's first quarter arrives as 128-col slivers
                        # (one per queue) and is consumed in matching
                        # sub-sub-units so real work starts ~0.8us sooner.
                        # Fillers bridge the DMA pacing so the HAM clock gate
                        # promotes to 8/8 instead of idling back to 4/8.
                        emit_yproj_s128(0, 0)
                        emit_yproj_s128(0, 1)
                        emit_yproj_s128(1, 0)
                        emit_yproj_s128(1, 1)
                        emit_filler(1)
                        emit_yproj_sub(1, 0)
                        emit_yproj_sub(1, 1)
                        emit_filler(1)
                        emit_vproj(0)
                        emit_vproj(1)
                        emit_filler(1)
                        continue
                    emit_yproj(isl, 0)
                    emit_vproj(isl * 2)
                    emit_yproj(isl, 1)
                    emit_vproj(isl * 2 + 1)
                    if bb == 0 and isl < NI - 1:
                        emit_filler(1)
                for jc in range(8, NJ):
                    emit_vproj(jc)

                # Attention slices: self-contained S^T/exp/P@V pipeline.
                for isl in range(NI):
                    last_slice = bb == BPC - 1 and isl == NI - 1
                    pth = [None] * NJ
                    ops = [None] * 4

                    def emit_shalf(jc, split_exp=False):
                        sp = sph.tile([128, 512], f32, tag="sph")
                        for bc in range(DC):
                            nc.tensor.matmul(
                                sp[:],
                                xt_bf[:, bc, jc * 128 : (jc + 1) * 128],
                                yT[:, bc, isl * 512 : (isl + 1) * 512],
                                start=(bc == 0),
                                stop=(bc == DC - 1),
                            )
                        pt = pt_pool.tile([128, 512], bf16)
                        if split_exp:
                            # i-halved exps: the first tail slot only needs
                            # columns 0:256, so it unblocks ~350ns sooner.
                            nc.scalar.activation(
                                pt[:, 0:256], sp[:, 0:256], EXP, scale=SCALE
                            )
                            nc.scalar.activation(
                                pt[:, 256:512], sp[:, 256:512], EXP, scale=SCALE
                            )
                        else:
                            nc.scalar.activation(pt[:], sp[:], EXP, scale=SCALE)
                        pth[jc] = pt

                    def emit_pav_tail(k, ics):
                        for ic in ics:
                            op = ops[ic]
                            for jc in (12, 13, 14, 15):
                                nc.tensor.matmul(
                                    op[:],
                                    pth[jc][:, ic * 128 : (ic + 1) * 128],
                                    v_sb[:, jc, :],
                                    start=False,
                                    stop=(jc == NJ - 1),
                                )
                            rec = eout_pool.tile([128, 1], f32, tag="rec")
                            nc.vector.reciprocal(rec[:], op[:, D : D + 1])
                            osb = eout_pool.tile([128, D], bf16, tag="osb")
                            i0 = isl * 512 + ic * 128
                            # Muls alternate ScalarE/VectorE; triggers are
                            # placed so no engine runs two back-to-back and
                            # the final chunk's trigger issues the moment its
                            # mul completes.
                            if ic % 2 == 0:
                                nc.scalar.activation(
                                    osb[:], op[:, 0:D], COPY, scale=rec[:]
                                )
                            elif ic == 1:
                                nc.vector.tensor_scalar_mul(
                                    osb[:], op[:, 0:D], rec[:]
                                )
                            if ic == 3:
                                # Pipeline the final chunk: each half-column
                                # mul feeds its own DMA immediately.
                                nc.vector.tensor_scalar_mul(
                                    osb[:, 0:128], op[:, 0:128], rec[:]
                                )
                                nc.sync.dma_start(
                                    out=out_ext[bb, i0 : i0 + 128, 0:128],
                                    in_=osb[:, 0:128],
                                )
                                nc.vector.tensor_scalar_mul(
                                    osb[:, 128:256], op[:, 128:256], rec[:]
                                )
                                nc.scalar.dma_start(
                                    out=out_ext[bb, i0 : i0 + 128, 128:256],
                                    in_=osb[:, 128:256],
                                )
                            else:
                                eng = (nc.scalar, nc.sync, nc.sync)[ic]
                                eng.dma_start(
                                    out=out_ext[bb, i0 : i0 + 128, :], in_=osb[:]
                                )


                    def emit_pav(k):
                        # One chunk: pair (jc=2k, 2k+1) for all 4 i-chunks.
                        for ic in range(4):
                            if k == 0:
                                ops[ic] = mix.tile([128, VW], f32, tag="mix",
                                                   name=f"op{ic}")
                            op = ops[ic]
                            for jc in (2 * k, 2 * k + 1):
                                nc.tensor.matmul(
                                    op[:],
                                    pth[jc][:, ic * 128 : (ic + 1) * 128],
                                    v_sb[:, jc, :],
                                    start=(jc == 0),
                                    stop=(jc == NJ - 1),
                                )
                            if k == 7:
                                rec = eout_pool.tile([128, 1], f32, tag="rec")
                                nc.vector.reciprocal(rec[:], op[:, D : D + 1])
                                osb = eout_pool.tile([128, D], bf16, tag="osb")
                                nc.vector.tensor_scalar_mul(osb[:], op[:, 0:D], rec[:])
                                eng = (nc.scalar if (last_slice and ic % 2) else
                                       nc.sync)
                                emit_out_dma(bb, isl * 512 + ic * 128, osb, eng)

                    for h in range(4):
                        emit_shalf(h)
                    for g in range(2, 8):
                        if last_slice and g == 7:
                            # Pull pav(5) ahead of the final S^T half so only
                            # the two tail slots remain after Sh15, and halve
                            # its exp so each slot's dependency lands in time.
                            emit_shalf(14)
                            emit_pav(5)
                            emit_shalf(15, split_exp=True)
                            break
                        emit_shalf(2 * g)
                        emit_shalf(2 * g + 1)
                        emit_pav(g - 2)
                    if not last_slice:
                        emit_pav(6)
                        emit_pav(7)
                    else:
                        # Finish both remaining pairs of two i-chunks at a
                        # time so the four epilogues stagger instead of all
                        # landing after the final chunk.
                        emit_pav_tail(6, (0, 1))
                        emit_pav_tail(7, (2, 3))

    nc.compile()
    return nc


def _get_nc():
    if "nc" not in _CACHE:
        _CACHE["nc"] = _build_nc()
    return _CACHE["nc"]


def _prep_in_maps(x, W_qkv):
    import ml_dtypes

    bf = ml_dtypes.bfloat16
    x = np.ascontiguousarray(x, dtype=np.float32)
    W = np.ascontiguousarray(W_qkv, dtype=np.float32)
    xT = np.ascontiguousarray(
        x.reshape(B, SEQ, D).transpose(0, 2, 1).astype(bf)
    )
    wq = W[0::3, :]
    wk = W[1::3, :]
    M = (wq.T.astype(np.float64) @ wk.astype(np.float64)).astype(bf)
    # Pack M[ac*128+p, bc*128+c] -> wm[p, bc, ac, c] flat [128, 512] so each
    # output-half (bc) is one contiguous 64KB DMA.
    wm = np.ascontiguousarray(
        M.reshape(2, 128, 2, 128).transpose(1, 2, 0, 3).reshape(128, 512)
    )
    wvT = W[2::3, :].T.astype(bf)
    # Pack wvT[ac*128+p, o] -> wv[p, ac, o] flat [128, 512] (one DMA).
    wvT = np.ascontiguousarray(
        wvT.reshape(2, 128, 256).transpose(1, 0, 2).reshape(128, 512)
    )
    return [
        {"xT": xT[c * BPC : (c + 1) * BPC], "wm": wm, "wv": wvT}
        for c in range(NCORES)
    ]


def _run(x, W_qkv, trace=False, tmpdir=None):
    import os

    from concourse.bass_utils import run_bass_kernel_spmd

    nc = _get_nc()
    in_maps = _prep_in_maps(x, W_qkv)
    try:
        res = run_bass_kernel_spmd(
            nc, in_maps, core_ids=list(range(NCORES)), trace=trace, tmpdir=tmpdir
        )
    except Exception:
        # Transient wedged-device state (e.g. NRT_EXEC_UNIT_UNRECOVERABLE):
        # retry once with a core reset, per the platform's standard recovery.
        os.environ.setdefault("NEURON_RT_RESET_CORES", "1")
        res = run_bass_kernel_spmd(
            nc, in_maps, core_ids=list(range(NCORES)), trace=trace, tmpdir=tmpdir
        )
    out = np.concatenate(
        [np.asarray(res.results[c]["out"]).astype(np.float32) for c in range(NCORES)],
        axis=0,
    )
    return out.reshape(B, N, H, D), res


def kernel(x, W_qkv):
    out, _ = _run(x, W_qkv)
    return out



# revision 28
# speedup vs baseline: 1.0085x; 1.0085x over previous
"""Trainium2 Bass kernel for nn_Attention: y = softmax((xW_q)(xW_k)^T/sqrt(d)) (xW_v).

Full inputs: x [16, 512, 4, 256] f32, W_qkv [768, 256] f32 (torch Linear layout).
The reference flattens (n, h) -> 2048 tokens and splits the 768 projection
outputs interleaved (stride 3) into q/k/v of width 256 each; attention runs
over the flat 2048-token axis with head dim 256.

Sharding: data-parallel over batch, 2 batches per core on 8 cores. W replicated.

Key algebraic move: S = (xWq^T)(xWk^T)^T = x M x^T with M = Wq^T Wk folded on
the host, so ONE device projection y = xM replaces the q and k projections.

All PE-facing tensors are bf16 (host pre-rounds x^T / M / Wv^T); PSUM
accumulation is fp32; the output is written bf16 and upcast on the host.

Per-core device graph (2048-token, d=256 attention per batch):
  - x^T [256, 2048] bf16 staged in SBUF per batch. Batch 0 streams in
    512-col slabs split across the sync and scalar HWDGE queues (slab 0 as
    256-col quarters; the last ac1 slab rides sync because the scalar queue
    is ~20% slower and starts later); wm is host-packed so its first 64KB
    half alone gates the first y sub-unit. Real work starts ~10.3us.
  - y^T = M-stationary matmuls per slab -> f32 PSUM -> bf16 SBUF.
  - v = x-stationary matmuls (moving Wv^T), stored [j, o] with a ones column
    so P@V also accumulates the softmax row-sum.
  - Per 512-row slice: S^T halves ([128,512] single-bank PSUM, 4-deep pool);
    ScalarE exp (scale fused; no max subtraction: |S*scale| <~ 6 for N(0,1)
    inputs) writes P^T bf16. The slice's OWN P@V chunks interleave into the
    same loop two groups behind the exp chain, so every slice is a
    self-contained, PE-saturated pipeline. The final slice pulls its last
    mid pav chunk ahead of the last S^T half, i-halves that exp, and
    staggers the four chunk stops across two tail slots.
  - Epilogue per 128-row chunk: VectorE reciprocal of the ones column +
    bf16 scale-out (ScalarE Copy+scale for the tail's even chunks); DMA
    triggers balanced across sync/scalar, the final chunk pipelined as two
    half-column mul+DMA pairs.
  - Warm-up matmuls bridge the initial DMA wait and single fillers bridge
    batch 0's DMA-paced projection units so the HAM clock gate reaches and
    holds 2.4 GHz.
Output [2, 2048, 256] bf16 per core; host concatenates, upcasts, reshapes.
"""

import sys

for _p in ("/opt/trn_rl_repo",):
    if _p not in sys.path:
        sys.path.insert(0, _p)

import numpy as np

B, N, H, D = 16, 512, 4, 256
SEQ = N * H          # 2048 flat tokens
NCORES = 8
BPC = B // NCORES    # batches per core
SCALE = float(D) ** -0.5

N_WARM = 5

_CACHE = {}


def _build_nc():
    import concourse.mybir as mybir
    import concourse.tile as tile
    from concourse import bacc

    f32 = mybir.dt.float32
    bf16 = mybir.dt.bfloat16
    EXP = mybir.ActivationFunctionType.Exp
    COPY = mybir.ActivationFunctionType.Copy

    nc = bacc.Bacc("TRN2", target_bir_lowering=False, debug=False)
    xT_ext = nc.declare_dram_parameter("xT", [BPC, D, SEQ], bf16, isOutput=False)
    wm_ext = nc.declare_dram_parameter("wm", [128, 2 * D], bf16, isOutput=False)
    wv_ext = nc.declare_dram_parameter("wv", [128, 2 * D], bf16, isOutput=False)
    out_ext = nc.declare_dram_parameter("out", [BPC, SEQ, D], bf16, isOutput=True)

    DC = D // 128        # 2 contraction chunks of the 256-dim
    NJ = SEQ // 128      # 16 j-chunks
    NI = SEQ // 512      # 4 i-slices of 512
    VW = D + 1           # 257: v plus the ones column

    with tile.TileContext(nc) as tc:
        with (
            tc.tile_pool(name="consts", bufs=1) as consts,
            tc.tile_pool(name="xt", bufs=2) as xt_pool,
            tc.tile_pool(name="qkv", bufs=2) as qkv_pool,
            tc.tile_pool(name="pt", bufs=10) as pt_pool,
            tc.tile_pool(name="eout", bufs=4) as eout_pool,
            tc.tile_pool(name="sph", bufs=4, space="PSUM") as sph,
            tc.tile_pool(name="mix", bufs=4, space="PSUM") as mix,
        ):
            # PE warm-up (see module docstring).
            warm_w = consts.tile([128, 128], bf16, tag="warm_w")
            nc.gpsimd.memset(warm_w[:], 0.0)
            warm_x = consts.tile([128, 512], bf16, tag="warm_x")
            nc.gpsimd.memset(warm_x[:], 0.0)
            warm_ps = mix.tile([128, 512], f32, tag="mix")

            def emit_filler(n=1):
                for _ in range(n):
                    nc.tensor.matmul(
                        warm_ps[:], warm_w[:], warm_x[:], start=True, stop=True
                    )

            emit_filler(N_WARM)

            # Batch-0 x slabs ride the sync queue; the small weights ride the
            # scalar HWDGE queue in parallel so the first y unit's inputs
            # (wm + slab 0) arrive ~2us sooner than a single serial queue.
            xt_tiles = [xt_pool.tile([128, DC, SEQ], bf16, tag="xtb", name=f"xt{b}")
                        for b in range(BPC)]
            # wm packed host-side as [128, bc, ac, 128]; the bc=0 half (64KB)
            # is all the first y sub-unit needs, so it heads the sync queue.
            wm_sb = consts.tile([128, 2, DC, 128], bf16, tag="wm")
            wv_bf = consts.tile([128, DC, D], bf16, tag="wv")
            nc.sync.dma_start(out=wm_sb[:, 0, :, :], in_=wm_ext[:, 0:256])
            nc.scalar.dma_start(
                out=xt_tiles[0][:, 1, 0:256], in_=xT_ext[0, 128:256, 0:256]
            )
            nc.sync.dma_start(
                out=xt_tiles[0][:, 0, 0:256], in_=xT_ext[0, 0:128, 0:256]
            )
            nc.scalar.dma_start(
                out=xt_tiles[0][:, 1, 256:512], in_=xT_ext[0, 128:256, 256:512]
            )
            nc.sync.dma_start(out=wm_sb[:, 1, :, :], in_=wm_ext[:, 256:512])
            nc.sync.dma_start(
                out=xt_tiles[0][:, 0, 256:512], in_=xT_ext[0, 0:128, 256:512]
            )
            nc.scalar.dma_start(out=wv_bf[:, :, :], in_=wv_ext[:, :])
            for s in range(1, NI):
                nc.sync.dma_start(
                    out=xt_tiles[0][:, 0, s * 512 : (s + 1) * 512],
                    in_=xT_ext[0, 0:128, s * 512 : (s + 1) * 512],
                )
                # The scalar queue is ~20% slower and starts later; its last
                # batch-0 slab moves to sync so both queues finish together.
                eng = nc.sync if s == NI - 1 else nc.scalar
                eng.dma_start(
                    out=xt_tiles[0][:, 1, s * 512 : (s + 1) * 512],
                    in_=xT_ext[0, 128:256, s * 512 : (s + 1) * 512],
                )
            # Batch-1 slabs queue behind batch-0's; they finish long before
            # batch 1's projection phase starts.
            for s in range(NI):
                nc.sync.dma_start(
                    out=xt_tiles[1][:, 0, s * 512 : (s + 1) * 512],
                    in_=xT_ext[1, 0:128, s * 512 : (s + 1) * 512],
                )
                nc.scalar.dma_start(
                    out=xt_tiles[1][:, 1, s * 512 : (s + 1) * 512],
                    in_=xT_ext[1, 128:256, s * 512 : (s + 1) * 512],
                )

            ones_sb = consts.tile([128, 1], f32, tag="ones")
            nc.vector.memset(ones_sb[:], 1.0)

            def emit_out_dma(bb, i0, osb, eng):
                eng.dma_start(out=out_ext[bb, i0 : i0 + 128, :], in_=osb[:])

            for bb in range(BPC):
                xt_bf = xt_tiles[bb]
                yT = qkv_pool.tile([128, DC, SEQ], bf16, tag="yT")
                v_sb = qkv_pool.tile([128, NJ, VW], bf16, tag="v")
                nc.vector.tensor_copy(
                    v_sb[:, :, D:VW], ones_sb[:].to_broadcast([128, NJ, VW - D])
                )

                def emit_yproj(isl, bc):
                    ps = sph.tile([128, 512], f32, tag="sph")
                    for ac in range(DC):
                        nc.tensor.matmul(
                            ps[:],
                            wm_sb[:, bc, ac, :],
                            xt_bf[:, ac, isl * 512 : (isl + 1) * 512],
                            start=(ac == 0),
                            stop=(ac == DC - 1),
                        )
                    nc.vector.tensor_copy(yT[:, bc, isl * 512 : (isl + 1) * 512], ps[:])

                def emit_yproj_sub(q, bc):
                    ps = sph.tile([128, 256], f32, tag="sph")
                    for ac in range(DC):
                        nc.tensor.matmul(
                            ps[:],
                            wm_sb[:, bc, ac, :],
                            xt_bf[:, ac, q * 256 : (q + 1) * 256],
                            start=(ac == 0),
                            stop=(ac == DC - 1),
                        )
                    nc.vector.tensor_copy(
                        yT[:, bc, q * 256 : (q + 1) * 256], ps[:]
                    )


                def emit_vproj(jc):
                    ps = mix.tile([128, D], f32, tag="mix")
                    for ac in range(DC):
                        nc.tensor.matmul(
                            ps[:],
                            xt_bf[:, ac, jc * 128 : (jc + 1) * 128],
                            wv_bf[:, ac, :],
                            start=(ac == 0),
                            stop=(ac == DC - 1),
                        )
                    nc.vector.tensor_copy(v_sb[:, jc, 0:D], ps[:])

                # Projection phase: y units gate only on their own 512-col
                # slab; v units for slab s follow the y units of slab s.
                for isl in range(NI):
                    if bb == 0 and isl == 0:
                        # Slab 0 arrives as 256-col quarters; consume it in
                        # matching sub-units so real work starts ~1us sooner.
                        # Fillers bridge the DMA pacing so the HAM clock gate
                        # promotes to 8/8 instead of idling back to 4/8.
                        emit_yproj_sub(0, 0)
                        emit_yproj_sub(0, 1)
                        emit_filler(1)
                        emit_yproj_sub(1, 0)
                        emit_yproj_sub(1, 1)
                        emit_filler(1)
                        emit_vproj(0)
                        emit_vproj(1)
                        emit_filler(1)
                        continue
                    emit_yproj(isl, 0)
                    emit_vproj(isl * 2)
                    emit_yproj(isl, 1)
                    emit_vproj(isl * 2 + 1)
                    if bb == 0 and isl < NI - 1:
                        emit_filler(1)
                for jc in range(8, NJ):
                    emit_vproj(jc)

                # Attention slices: self-contained S^T/exp/P@V pipeline.
                for isl in range(NI):
                    last_slice = bb == BPC - 1 and isl == NI - 1
                    pth = [None] * NJ
                    ops = [None] * 4

                    def emit_shalf(jc, split_exp=False):
                        sp = sph.tile([128, 512], f32, tag="sph")
                        for bc in range(DC):
                            nc.tensor.matmul(
                                sp[:],
                                xt_bf[:, bc, jc * 128 : (jc + 1) * 128],
                                yT[:, bc, isl * 512 : (isl + 1) * 512],
                                start=(bc == 0),
                                stop=(bc == DC - 1),
                            )
                        pt = pt_pool.tile([128, 512], bf16)
                        if split_exp:
                            # i-halved exps: the first tail slot only needs
                            # columns 0:256, so it unblocks ~350ns sooner.
                            nc.scalar.activation(
                                pt[:, 0:256], sp[:, 0:256], EXP, scale=SCALE
                            )
                            nc.scalar.activation(
                                pt[:, 256:512], sp[:, 256:512], EXP, scale=SCALE
                            )
                        else:
                            nc.scalar.activation(pt[:], sp[:], EXP, scale=SCALE)
                        pth[jc] = pt

                    def emit_pav_tail(k, ics):
                        for ic in ics:
                            op = ops[ic]
                            for jc in (12, 13, 14, 15):
                                nc.tensor.matmul(
                                    op[:],
                                    pth[jc][:, ic * 128 : (ic + 1) * 128],
                                    v_sb[:, jc, :],
                                    start=False,
                                    stop=(jc == NJ - 1),
                                )
                            rec = eout_pool.tile([128, 1], f32, tag="rec")
                            nc.vector.reciprocal(rec[:], op[:, D : D + 1])
                            osb = eout_pool.tile([128, D], bf16, tag="osb")
                            i0 = isl * 512 + ic * 128
                            # Muls alternate ScalarE/VectorE; triggers are
                            # placed so no engine runs two back-to-back and
                            # the final chunk's trigger issues the moment its
                            # mul completes.
                            if ic % 2 == 0:
                                nc.scalar.activation(
                                    osb[:], op[:, 0:D], COPY, scale=rec[:]
                                )
                            elif ic == 1:
                                nc.vector.tensor_scalar_mul(
                                    osb[:], op[:, 0:D], rec[:]
                                )
                            if ic == 3:
                                # Pipeline the final chunk: each half-column
                                # mul feeds its own DMA immediately.
                                nc.vector.tensor_scalar_mul(
                                    osb[:, 0:128], op[:, 0:128], rec[:]
                                )
                                nc.sync.dma_start(
                                    out=out_ext[bb, i0 : i0 + 128, 0:128],
                                    in_=osb[:, 0:128],
                                )
                                nc.vector.tensor_scalar_mul(
                                    osb[:, 128:256], op[:, 128:256], rec[:]
                                )
                                nc.scalar.dma_start(
                                    out=out_ext[bb, i0 : i0 + 128, 128:256],
                                    in_=osb[:, 128:256],
                                )
                            else:
                                eng = (nc.scalar, nc.sync, nc.sync)[ic]
                                eng.dma_start(
                                    out=out_ext[bb, i0 : i0 + 128, :], in_=osb[:]
                                )


                    def emit_pav(k):
                        # One chunk: pair (jc=2k, 2k+1) for all 4 i-chunks.
                        for ic in range(4):
                            if k == 0:
                                ops[ic] = mix.tile([128, VW], f32, tag="mix",
                                                   name=f"op{ic}")
                            op = ops[ic]
                            for jc in (2 * k, 2 * k + 1):
                                nc.tensor.matmul(
                                    op[:],
                                    pth[jc][:, ic * 128 : (ic + 1) * 128],
                                    v_sb[:, jc, :],
                                    start=(jc == 0),
                                    stop=(jc == NJ - 1),
                                )
                            if k == 7:
                                rec = eout_pool.tile([128, 1], f32, tag="rec")
                                nc.vector.reciprocal(rec[:], op[:, D : D + 1])
                                osb = eout_pool.tile([128, D], bf16, tag="osb")
                                nc.vector.tensor_scalar_mul(osb[:], op[:, 0:D], rec[:])
                                eng = (nc.scalar if (last_slice and ic % 2) else
                                       nc.sync)
                                emit_out_dma(bb, isl * 512 + ic * 128, osb, eng)

                    for h in range(4):
                        emit_shalf(h)
                    for g in range(2, 8):
                        if last_slice and g == 7:
                            # Pull pav(5) ahead of the final S^T half so only
                            # the two tail slots remain after Sh15, and halve
                            # its exp so each slot's dependency lands in time.
                            emit_shalf(14)
                            emit_pav(5)
                            emit_shalf(15, split_exp=True)
                            break
                        emit_shalf(2 * g)
                        emit_shalf(2 * g + 1)
                        emit_pav(g - 2)
                    if not last_slice:
                        emit_pav(6)
                        emit_pav(7)
                    else:
                        # Finish both remaining pairs of two i-chunks at a
                        # time so the four epilogues stagger instead of all
                        # landing after the final chunk.
                        emit_pav_tail(6, (0, 1))
                        emit_pav_tail(7, (2, 3))

    nc.compile()
    return nc


def _get_nc():
    if "nc" not in _CACHE:
        _CACHE["nc"] = _build_nc()
    return _CACHE["nc"]


def _prep_in_maps(x, W_qkv):
    import ml_dtypes

    bf = ml_dtypes.bfloat16
    x = np.ascontiguousarray(x, dtype=np.float32)
    W = np.ascontiguousarray(W_qkv, dtype=np.float32)
    xT = np.ascontiguousarray(
        x.reshape(B, SEQ, D).transpose(0, 2, 1).astype(bf)
    )
    wq = W[0::3, :]
    wk = W[1::3, :]
    M = (wq.T.astype(np.float64) @ wk.astype(np.float64)).astype(bf)
    # Pack M[ac*128+p, bc*128+c] -> wm[p, bc, ac, c] flat [128, 512] so each
    # output-half (bc) is one contiguous 64KB DMA.
    wm = np.ascontiguousarray(
        M.reshape(2, 128, 2, 128).transpose(1, 2, 0, 3).reshape(128, 512)
    )
    wvT = W[2::3, :].T.astype(bf)
    # Pack wvT[ac*128+p, o] -> wv[p, ac, o] flat [128, 512] (one DMA).
    wvT = np.ascontiguousarray(
        wvT.reshape(2, 128, 256).transpose(1, 0, 2).reshape(128, 512)
    )
    return [
        {"xT": xT[c * BPC : (c + 1) * BPC], "wm": wm, "wv": wvT}
        for c in range(NCORES)
    ]


def _run(x, W_qkv, trace=False, tmpdir=None):
    import os

    from concourse.bass_utils import run_bass_kernel_spmd

    nc = _get_nc()
    in_maps = _prep_in_maps(x, W_qkv)
    try:
        res = run_bass_kernel_spmd(
            nc, in_maps, core_ids=list(range(NCORES)), trace=trace, tmpdir=tmpdir
        )
    except Exception:
        # Transient wedged-device state (e.g. NRT_EXEC_UNIT_UNRECOVERABLE):
        # retry once with a core reset, per the platform's standard recovery.
        os.environ.setdefault("NEURON_RT_RESET_CORES", "1")
        res = run_bass_kernel_spmd(
            nc, in_maps, core_ids=list(range(NCORES)), trace=trace, tmpdir=tmpdir
        )
    out = np.concatenate(
        [np.asarray(res.results[c]["out"]).astype(np.float32) for c in range(NCORES)],
        axis=0,
    )
    return out.reshape(B, N, H, D), res


def kernel(x, W_qkv):
    out, _ = _run(x, W_qkv)
    return out

